# Initial kernel scaffold
#
"""Trainium2 Bass kernel for nn_Attention (pooling attention).

Math (per batch b):
    u[b]     = W_score @ h_t[b]            (tiny: score = (hidden @ W_score) . h_t
                                            collapses to hidden . (W_score @ h_t))
    score[t] = hidden[b,t,:] . u[b]        (DVE fp16 mul + pairwise-add tree)
    p[t]     = exp(score[t] - 50)          (ScalarE, fused per-partition accum -> q)
    s        = sum_t p[t]                  (PE ones-matmul over q)
    w[t]     = p[t] / s                    (normalized weights, fp16-safe)
    ctx      = sum_t w[t] * hidden[b,t,:]  (PE fp16: w column as 1-col stationary)
    out[b]   = tanh([ctx, h_t[b]] @ W_att)

Sharding: data-parallel over batch, 16 batches per core on 8 cores; weights
replicated.  hidden_states is read from HBM exactly once (fp32), cast to fp16
during the DMA (SWDGE cast), and never transposed.
"""

import sys

import numpy as np

_TRN_REPO = "/opt/trn_rl_repo"
if _TRN_REPO not in sys.path:
    sys.path.insert(0, _TRN_REPO)

import concourse.bass as bass
import concourse.bacc as bacc
import concourse.tile as tile
from concourse import mybir
from concourse import bass_isa
from concourse.bass_utils import run_bass_kernel_spmd

N_CORES = 8
B, T, H = 128, 2048, 256
NB = B // N_CORES  # batches per core
P = 128  # SBUF partitions
TT = T // P  # t-tiles per batch
OUT_D = 128
EXP_SHIFT = -50.0  # keeps exp() in fp32 range; cancels in the softmax ratio

CAST_DMA = True  # fp32->fp16 during the load DMA (SWDGE); else ACT casts

F32 = mybir.dt.float32
F16 = mybir.dt.float16


def _build_kernel(nc: bass.Bass, tc: "tile.TileContext", hidden, wst, watt, ident, out):
    add = mybir.AluOpType.add

    from contextlib import ExitStack

    with ExitStack() as ctx:
        const = ctx.enter_context(tc.tile_pool(name="const", bufs=1))
        ybufs = ctx.enter_context(tc.tile_pool(name="ybufs", bufs=7))
        sc = ctx.enter_context(tc.tile_pool(name="sc", bufs=3))
        psum_t = ctx.enter_context(tc.tile_pool(name="psum_t", bufs=2, space="PSUM"))
        psum_p = ctx.enter_context(tc.tile_pool(name="psum_p", bufs=1, space="PSUM"))
        dram = ctx.enter_context(tc.tile_pool(name="dram", bufs=1, space="DRAM"))

        # ---- constants / weights -------------------------------------------------
        # Setup DMAs ride the sync (HWDGE) queue; the gpsimd (SWDGE) queue
        # carries only the 16 big y16 cast-loads.
        # issue order matters: everything the u-chain needs (ident, ht, wst)
        # must complete BEFORE the y16 load flood saturates the SDMAs; watt is
        # only read by the final matmuls and may be starved without harm.
        ident_sb = const.tile([16, 16], F32, tag="ident")
        nc.sync.dma_start(out=ident_sb, in_=ident[:, :])
        ht_early = const.tile([NB, H], F32, tag="ht")
        ht_dma = nc.sync.dma_start(out=ht_early, in_=hidden[:, T - 1, :])
        wst_sb = const.tile([P, 2, H], F32, tag="wst")  # W_score^T as [k, kk, h]
        wst_dma = nc.sync.dma_start(
            out=wst_sb, in_=wst.rearrange("(kk p) h -> p kk h", p=P)
        )
        watt_sb = const.tile([P, 4, OUT_D], F32, tag="watt")  # W_att as [d, dd, j]
        ones_col = const.tile([P, 1], F32, tag="ones_col")
        nc.vector.memset(ones_col, 1.0)
        ones_row = const.tile([1, P], F32, tag="ones_row")
        nc.vector.memset(ones_row, 1.0)
        shift_col = const.tile([P, 1], F32, tag="shift_col")
        nc.vector.memset(shift_col, EXP_SHIFT)

        # ---- h_t, h_t^T and u = h_t @ W_score^T ---------------------------------
        ht_sb = ht_early
        htT_sb = const.tile([P, 2, NB], F32, tag="htT")  # h_t^T halves [k, half, b]
        for half in range(2):
            ps_tr = psum_t.tile([P, NB], F32, tag="ptmp", name=f"ps_tr{half}")
            nc.tensor.matmul(
                ps_tr,
                lhsT=ht_sb[:, half * P : (half + 1) * P],
                rhs=ident_sb,
                start=True,
                stop=True,
            )
            nc.scalar.copy(out=htT_sb[:, half, :], in_=ps_tr)

        ps_u = psum_t.tile([NB, H], F32, tag="ptmp")
        for half in range(2):
            nc.tensor.matmul(
                ps_u,
                lhsT=htT_sb[:, half, :],
                rhs=wst_sb[:, half, :],
                start=(half == 0),
                stop=(half == 1),
            )
        u16_sb = const.tile([NB, H], F16, tag="u16")
        nc.scalar.copy(out=u16_sb, in_=ps_u)

        # per-batch broadcast of u[b] to all 128 partitions: SBUF->SBUF DMA with
        # a zero-stride REP dim on the free axis of the single-partition source
        # (partition-dim stride stays nonzero, so the AP is legal). No DRAM
        # bounce, no HBM contention with the y16 load flood.
        ubc_all = const.tile([P, NB, H], F16, tag="ubc_all")
        for b in range(NB):
            row = u16_sb[b : b + 1, :]
            ubc_src = bass.AP(
                tensor=row.tensor,
                offset=row.offset,
                ap=[list(row.ap[0]), [0, P]] + [list(x) for x in row.ap[1:]],
            )
            nc.sync.dma_start(out=ubc_all[:, b, :], in_=ubc_src)
        nc.sync.dma_start(out=watt_sb, in_=watt.rearrange("(dd p) j -> p dd j", p=P))

        # ---- persistent PSUM accumulators ---------------------------------------
        ctxT_ps = [
            psum_p.tile([P, NB], F32, tag=f"ctxT{j}", name=f"ctxT{j}")
            for j in range(2)
        ]

        # ---- per-batch pipeline --------------------------------------------------
        # t = p*TT + i block mapping gives 16KB-contiguous runs per partition
        # (softmax/context are t-permutation-invariant, so relabeling is free).
        # software prefetch: emit each load PREF batches ahead so the
        # per-batch PARTITION_ALL_REDUCE (same gpsimd FIFO, waits on exp)
        # never head-of-line-blocks a load descgen the pipeline needs soon.
        PREF = 3
        ylist = {}

        def emit_load(k):
            y = ybufs.tile([P, TT, H], F16, tag="y16", name=f"y16_{k}")
            yld = nc.gpsimd.dma_start(
                out=y, in_=hidden[k].rearrange("(p i) h -> p i h", i=TT)
            )
            if k == 0:
                # hold the load flood until the setup prologue's transfers are
                # done (sync ring is FIFO, so ht-done implies ident done);
                # otherwise the u-chain starves ~20us behind the flood.
                tile.add_dep_helper(
                    yld.ins, ht_dma.ins, reason="flood waits for setup DMAs"
                )
            ylist[k] = y

        for k in range(min(PREF, NB)):
            emit_load(k)

        for b in range(NB):
            if b + PREF < NB:
                emit_load(b + PREF)
            y16 = ylist.pop(b)

            # score[t] = y[t, :] . u : fp16 mul + pairwise tree + fp32 reduce
            z = sc.tile([P, TT, H], F16, tag="z")
            ubc = ubc_all[:, b, :]
            ubc_rep = bass.AP(
                tensor=ubc.tensor,
                offset=ubc.offset,
                ap=[list(ubc.ap[0]), [0, TT], list(ubc.ap[1])],
            )
            nc.vector.tensor_mul(z, y16, ubc_rep)
            z1 = sc.tile([P, TT, 128], F16, tag="z1")
            nc.vector.tensor_add(z1, z[:, :, 0:128], z[:, :, 128:256])
            z2 = sc.tile([P, TT, 64], F16, tag="z2")
            nc.vector.tensor_add(z2, z1[:, :, 0:64], z1[:, :, 64:128])
            score = sc.tile([P, TT], F32, tag="score")
            nc.vector.tensor_reduce(
                out=score, in_=z2, axis=mybir.AxisListType.X, op=add
            )

            # p = exp(score - 50), q = per-partition sum of p
            p_t = sc.tile([P, TT], F32, tag="p")
            q = sc.tile([P, 1], F32, tag="q")
            nc.scalar.activation(
                out=p_t,
                in_=score,
                func=mybir.ActivationFunctionType.Exp,
                bias=shift_col,
                scale=1.0,
                accum_out=q,
            )

            # s = sum over partitions of q, landed on every partition by the
            # GPSIMD daisy chain (no PE/ACT round trips on the critical path)
            s_bc = sc.tile([P, 1], F32, tag="s_bc")
            nc.gpsimd.partition_all_reduce(s_bc, q, P, bass_isa.ReduceOp.add)
            rs_bc = sc.tile([P, 1], F32, tag="rs_bc")
            nc.vector.reciprocal(out=rs_bc, in_=s_bc)

            # w = p / s  (normalized, fp16-safe)
            w16 = sc.tile([P, TT], F16, tag="w16")
            nc.vector.tensor_scalar_mul(out=w16, in0=p_t, scalar1=rs_bc)

            # ctx_row = sum_t w[t] * y[t, :]  (fp16 matmuls, accumulate 16 t-tiles)
            ctx_ps = psum_t.tile([1, H], F32, tag="ptmp")
            for i in range(TT):
                nc.tensor.matmul(
                    ctx_ps,
                    lhsT=w16[:, i : i + 1],
                    rhs=y16[:, i, :],
                    start=(i == 0),
                    stop=(i == TT - 1),
                )
            ctx_row = sc.tile([1, H], F32, tag="ctx_row_sb")
            nc.scalar.copy(out=ctx_row, in_=ctx_ps)

            # scatter ctx_row into column b of the persistent ctx^T accumulators
            for j in range(2):
                nc.tensor.matmul(
                    ctxT_ps[j][:, b : b + 1],
                    lhsT=ctx_row[:, j * P : (j + 1) * P],
                    rhs=ones_row[:, 0:1],
                    start=True,
                    stop=True,
                )

        # ---- finalize: concat with h_t, @W_att, tanh ----------------------------
        preT = sc.tile([P, 2, NB], F32, tag="preT")
        for j in range(2):
            nc.scalar.copy(out=preT[:, j, :], in_=ctxT_ps[j])

        out_ps = psum_t.tile([NB, OUT_D], F32, tag="ptmp")
        for dd in range(4):
            lhsT = preT[:, dd, :] if dd < 2 else htT_sb[:, dd - 2, :]
            nc.tensor.matmul(
                out_ps,
                lhsT=lhsT,
                rhs=watt_sb[:, dd, :],
                start=(dd == 0),
                stop=(dd == 3),
            )
        out_sb = sc.tile([NB, OUT_D], F32, tag="out_sb")
        nc.scalar.activation(
            out=out_sb, in_=out_ps, func=mybir.ActivationFunctionType.Tanh
        )
        nc.sync.dma_start(out=out[:, :], in_=out_sb)


_NC_CACHE = {}


def _get_nc():
    if "nc" not in _NC_CACHE:
        nc = bacc.Bacc("TRN2", target_bir_lowering=False, debug=False)
        hidden = nc.declare_dram_parameter("hidden", [NB, T, H], F32, isOutput=False)
        wst = nc.declare_dram_parameter("w_score_t", [H, H], F32, isOutput=False)
        watt = nc.declare_dram_parameter("w_att", [2 * H, OUT_D], F32, isOutput=False)
        ident = nc.declare_dram_parameter("ident16", [16, 16], F32, isOutput=False)
        out = nc.declare_dram_parameter("out", [NB, OUT_D], F32, isOutput=True)
        with tile.TileContext(nc) as tc:
            _build_kernel(nc, tc, hidden, wst, watt, ident, out)
        nc.compile()
        _NC_CACHE["nc"] = nc
    return _NC_CACHE["nc"]


def _run(hidden_states, W_score, W_att, trace=False, trace_kwargs=None):
    hidden_states = np.ascontiguousarray(np.asarray(hidden_states, dtype=np.float32))
    W_score = np.asarray(W_score, dtype=np.float32)
    W_att = np.ascontiguousarray(np.asarray(W_att, dtype=np.float32))
    wst = np.ascontiguousarray(W_score.T)
    ident = np.eye(16, dtype=np.float32)

    nc = _get_nc()
    in_maps = []
    for c in range(N_CORES):
        in_maps.append(
            {
                "hidden": hidden_states[c * NB : (c + 1) * NB],
                "w_score_t": wst,
                "w_att": W_att,
                "ident16": ident,
            }
        )
    kwargs = {}
    if trace:
        kwargs["trace"] = True
        if trace_kwargs:
            kwargs.update(trace_kwargs)
    res = run_bass_kernel_spmd(nc, in_maps, list(range(N_CORES)), **kwargs)
    out = np.concatenate([res.results[c]["out"] for c in range(N_CORES)], axis=0)
    return out, res


def kernel(hidden_states, W_score, W_att):
    out, _ = _run(hidden_states, W_score, W_att, trace=False)
    return out



# revision 1
# speedup vs baseline: 2.4925x; 2.4925x over previous
"""Trainium2 Bass kernel for nn_Attention (pooling attention).

Math (per batch b):
    u[b]     = W_score @ h_t[b]            (tiny: score = (hidden @ W_score) . h_t
                                            collapses to hidden . (W_score @ h_t))
    score[t] = hidden[b,t,:] . u[b]        (DVE fp16 mul + pairwise-add tree)
    p[t]     = exp(score[t] - 50)          (ScalarE, fused per-partition accum -> q)
    s        = sum_t p[t]                  (PE ones-matmul over q)
    w[t]     = p[t] / s                    (normalized weights, fp16-safe)
    ctx      = sum_t w[t] * hidden[b,t,:]  (PE fp16: w column as 1-col stationary)
    out[b]   = tanh([ctx, h_t[b]] @ W_att)

Sharding: data-parallel over batch, 16 batches per core on 8 cores; weights
replicated.  hidden_states is read from HBM exactly once (fp32), cast to fp16
during the DMA (SWDGE cast), and never transposed.
"""

import sys

import numpy as np

_TRN_REPO = "/opt/trn_rl_repo"
if _TRN_REPO not in sys.path:
    sys.path.insert(0, _TRN_REPO)

import concourse.bass as bass
import concourse.bacc as bacc
import concourse.tile as tile
from concourse import mybir
from concourse import bass_isa
from concourse.bass_utils import run_bass_kernel_spmd

N_CORES = 8
B, T, H = 128, 2048, 256
NB = B // N_CORES  # batches per core
P = 128  # SBUF partitions
TT = T // P  # t-tiles per batch
OUT_D = 128
EXP_SHIFT = -50.0  # keeps exp() in fp32 range; cancels in the softmax ratio

CAST_DMA = True  # fp32->fp16 during the load DMA (SWDGE); else ACT casts

F32 = mybir.dt.float32
F16 = mybir.dt.float16


def _build_kernel(nc: bass.Bass, tc: "tile.TileContext", hidden, wst, watt, ident, out):
    add = mybir.AluOpType.add

    from contextlib import ExitStack

    with ExitStack() as ctx:
        const = ctx.enter_context(tc.tile_pool(name="const", bufs=1))
        ybufs = ctx.enter_context(tc.tile_pool(name="ybufs", bufs=7))
        sc = ctx.enter_context(tc.tile_pool(name="sc", bufs=3))
        psum_t = ctx.enter_context(tc.tile_pool(name="psum_t", bufs=2, space="PSUM"))
        psum_p = ctx.enter_context(tc.tile_pool(name="psum_p", bufs=1, space="PSUM"))
        dram = ctx.enter_context(tc.tile_pool(name="dram", bufs=1, space="DRAM"))

        # ---- constants / weights -------------------------------------------------
        # Setup DMAs ride the sync (HWDGE) queue; the gpsimd (SWDGE) queue
        # carries only the 16 big y16 cast-loads.
        # issue order matters: everything the u-chain needs (ident, ht, wst)
        # must complete BEFORE the y16 load flood saturates the SDMAs; watt is
        # only read by the final matmuls and may be starved without harm.
        ident_sb = const.tile([16, 16], F32, tag="ident")
        nc.sync.dma_start(out=ident_sb, in_=ident[:, :])
        ht_early = const.tile([NB, H], F32, tag="ht")
        ht_dma = nc.sync.dma_start(out=ht_early, in_=hidden[:, T - 1, :])
        wst_sb = const.tile([P, 2, H], F32, tag="wst")  # W_score^T as [k, kk, h]
        wst_dma = nc.sync.dma_start(
            out=wst_sb, in_=wst.rearrange("(kk p) h -> p kk h", p=P)
        )
        watt_sb = const.tile([P, 4, OUT_D], F32, tag="watt")  # W_att as [d, dd, j]
        ones_col = const.tile([P, 1], F32, tag="ones_col")
        nc.vector.memset(ones_col, 1.0)
        ones_row = const.tile([1, P], F32, tag="ones_row")
        nc.vector.memset(ones_row, 1.0)
        shift_col = const.tile([P, 1], F32, tag="shift_col")
        nc.vector.memset(shift_col, EXP_SHIFT)

        # ---- h_t, h_t^T and u = h_t @ W_score^T ---------------------------------
        ht_sb = ht_early
        htT_sb = const.tile([P, 2, NB], F32, tag="htT")  # h_t^T halves [k, half, b]
        for half in range(2):
            ps_tr = psum_t.tile([P, NB], F32, tag="ptmp", name=f"ps_tr{half}")
            nc.tensor.matmul(
                ps_tr,
                lhsT=ht_sb[:, half * P : (half + 1) * P],
                rhs=ident_sb,
                start=True,
                stop=True,
            )
            nc.scalar.copy(out=htT_sb[:, half, :], in_=ps_tr)

        ps_u = psum_t.tile([NB, H], F32, tag="ptmp")
        for half in range(2):
            nc.tensor.matmul(
                ps_u,
                lhsT=htT_sb[:, half, :],
                rhs=wst_sb[:, half, :],
                start=(half == 0),
                stop=(half == 1),
            )
        u16_sb = const.tile([NB, H], F16, tag="u16")
        nc.scalar.copy(out=u16_sb, in_=ps_u)

        # per-batch broadcast of u[b] to all 128 partitions: SBUF->SBUF DMA with
        # a zero-stride REP dim on the free axis of the single-partition source
        # (partition-dim stride stays nonzero, so the AP is legal). No DRAM
        # bounce, no HBM contention with the y16 load flood.
        ubc_all = const.tile([P, NB, H], F16, tag="ubc_all")
        for b in range(NB):
            row = u16_sb[b : b + 1, :]
            ubc_src = bass.AP(
                tensor=row.tensor,
                offset=row.offset,
                ap=[list(row.ap[0]), [0, P]] + [list(x) for x in row.ap[1:]],
            )
            nc.sync.dma_start(out=ubc_all[:, b, :], in_=ubc_src)
        nc.sync.dma_start(out=watt_sb, in_=watt.rearrange("(dd p) j -> p dd j", p=P))

        # ---- persistent PSUM accumulators ---------------------------------------
        ctxT_ps = [
            psum_p.tile([P, NB], F32, tag=f"ctxT{j}", name=f"ctxT{j}")
            for j in range(2)
        ]

        # ---- per-batch pipeline --------------------------------------------------
        # t = p*TT + i block mapping gives 16KB-contiguous runs per partition
        # (softmax/context are t-permutation-invariant, so relabeling is free).
        # software prefetch: emit each load PREF batches ahead so the
        # per-batch PARTITION_ALL_REDUCE (same gpsimd FIFO, waits on exp)
        # never head-of-line-blocks a load descgen the pipeline needs soon.
        PREF = 3
        ylist = {}

        def emit_load(k):
            y = ybufs.tile([P, TT, H], F16, tag="y16", name=f"y16_{k}")
            yld = nc.gpsimd.dma_start(
                out=y, in_=hidden[k].rearrange("(p i) h -> p i h", i=TT)
            )
            if k == 0:
                # hold the load flood until the setup prologue's transfers are
                # done (sync ring is FIFO, so ht-done implies ident done);
                # otherwise the u-chain starves ~20us behind the flood.
                tile.add_dep_helper(
                    yld.ins, ht_dma.ins, reason="flood waits for setup DMAs"
                )
            ylist[k] = y

        for k in range(min(PREF, NB)):
            emit_load(k)

        for b in range(NB):
            if b + PREF < NB:
                emit_load(b + PREF)
            y16 = ylist.pop(b)

            # score[t] = y[t, :] . u : fp16 mul + pairwise tree + fp32 reduce
            z = sc.tile([P, TT, H], F16, tag="z")
            ubc = ubc_all[:, b, :]
            ubc_rep = bass.AP(
                tensor=ubc.tensor,
                offset=ubc.offset,
                ap=[list(ubc.ap[0]), [0, TT], list(ubc.ap[1])],
            )
            nc.vector.tensor_mul(z, y16, ubc_rep)
            z1 = sc.tile([P, TT, 128], F16, tag="z1")
            nc.vector.tensor_add(z1, z[:, :, 0:128], z[:, :, 128:256])
            z2 = sc.tile([P, TT, 64], F16, tag="z2")
            nc.vector.tensor_add(z2, z1[:, :, 0:64], z1[:, :, 64:128])
            score = sc.tile([P, TT], F32, tag="score")
            nc.vector.tensor_reduce(
                out=score, in_=z2, axis=mybir.AxisListType.X, op=add
            )

            # p = exp(score - 50), q = per-partition sum of p
            p_t = sc.tile([P, TT], F32, tag="p")
            q = sc.tile([P, 1], F32, tag="q")
            nc.scalar.activation(
                out=p_t,
                in_=score,
                func=mybir.ActivationFunctionType.Exp,
                bias=shift_col,
                scale=1.0,
                accum_out=q,
            )

            # s = sum over partitions of q, landed on every partition by the
            # GPSIMD daisy chain (no PE/ACT round trips on the critical path)
            s_bc = sc.tile([P, 1], F32, tag="s_bc")
            nc.gpsimd.partition_all_reduce(s_bc, q, P, bass_isa.ReduceOp.add)
            rs_bc = sc.tile([P, 1], F32, tag="rs_bc")
            nc.vector.reciprocal(out=rs_bc, in_=s_bc)

            # w = p / s  (normalized, fp16-safe)
            w16 = sc.tile([P, TT], F16, tag="w16")
            nc.vector.tensor_scalar_mul(out=w16, in0=p_t, scalar1=rs_bc)

            # ctx_row = sum_t w[t] * y[t, :]  (fp16 matmuls, accumulate 16 t-tiles)
            ctx_ps = psum_t.tile([1, H], F32, tag="ptmp")
            for i in range(TT):
                nc.tensor.matmul(
                    ctx_ps,
                    lhsT=w16[:, i : i + 1],
                    rhs=y16[:, i, :],
                    start=(i == 0),
                    stop=(i == TT - 1),
                )
            ctx_row = sc.tile([1, H], F32, tag="ctx_row_sb")
            nc.scalar.copy(out=ctx_row, in_=ctx_ps)

            # scatter ctx_row into column b of the persistent ctx^T accumulators
            for j in range(2):
                nc.tensor.matmul(
                    ctxT_ps[j][:, b : b + 1],
                    lhsT=ctx_row[:, j * P : (j + 1) * P],
                    rhs=ones_row[:, 0:1],
                    start=True,
                    stop=True,
                )

        # ---- finalize: concat with h_t, @W_att, tanh ----------------------------
        preT = sc.tile([P, 2, NB], F32, tag="preT")
        for j in range(2):
            nc.scalar.copy(out=preT[:, j, :], in_=ctxT_ps[j])

        out_ps = psum_t.tile([NB, OUT_D], F32, tag="ptmp")
        for dd in range(4):
            lhsT = preT[:, dd, :] if dd < 2 else htT_sb[:, dd - 2, :]
            nc.tensor.matmul(
                out_ps,
                lhsT=lhsT,
                rhs=watt_sb[:, dd, :],
                start=(dd == 0),
                stop=(dd == 3),
            )
        out_sb = sc.tile([NB, OUT_D], F32, tag="out_sb")
        nc.scalar.activation(
            out=out_sb, in_=out_ps, func=mybir.ActivationFunctionType.Tanh
        )
        nc.sync.dma_start(out=out[:, :], in_=out_sb)


_NC_CACHE = {}


def _get_nc():
    if "nc" not in _NC_CACHE:
        nc = bacc.Bacc("TRN2", target_bir_lowering=False, debug=False)
        hidden = nc.declare_dram_parameter("hidden", [NB, T, H], F32, isOutput=False)
        wst = nc.declare_dram_parameter("w_score_t", [H, H], F32, isOutput=False)
        watt = nc.declare_dram_parameter("w_att", [2 * H, OUT_D], F32, isOutput=False)
        ident = nc.declare_dram_parameter("ident16", [16, 16], F32, isOutput=False)
        out = nc.declare_dram_parameter("out", [NB, OUT_D], F32, isOutput=True)
        with tile.TileContext(nc) as tc:
            _build_kernel(nc, tc, hidden, wst, watt, ident, out)
        nc.compile()
        _NC_CACHE["nc"] = nc
    return _NC_CACHE["nc"]


def _run(hidden_states, W_score, W_att, trace=False, trace_kwargs=None):
    hidden_states = np.ascontiguousarray(np.asarray(hidden_states, dtype=np.float32))
    W_score = np.asarray(W_score, dtype=np.float32)
    W_att = np.ascontiguousarray(np.asarray(W_att, dtype=np.float32))
    wst = np.ascontiguousarray(W_score.T)
    ident = np.eye(16, dtype=np.float32)

    nc = _get_nc()
    in_maps = []
    for c in range(N_CORES):
        in_maps.append(
            {
                "hidden": hidden_states[c * NB : (c + 1) * NB],
                "w_score_t": wst,
                "w_att": W_att,
                "ident16": ident,
            }
        )
    kwargs = {}
    if trace:
        kwargs["trace"] = True
        if trace_kwargs:
            kwargs.update(trace_kwargs)
    res = run_bass_kernel_spmd(nc, in_maps, list(range(N_CORES)), **kwargs)
    out = np.concatenate([res.results[c]["out"] for c in range(N_CORES)], axis=0)
    return out, res


def kernel(hidden_states, W_score, W_att):
    out, _ = _run(hidden_states, W_score, W_att, trace=False)
    return out

